# revision 1
# baseline (speedup 1.0000x reference)
"""Trainium2 Bass kernel for LocallyConnected1D (filters=1, k=1, no bias):

    out[b, s, 0] = sum_c x[b, s, c] * W[s, c]

x: (256, 8192, 64) f32, W: (8192, 64) f32, out: (256, 8192, 1) f32.

Strategy: pure data-parallel over batch across the 8 NeuronCores (32
batches/core, W replicated, no collectives).  Per core, x[b] (2 MB) and W
(2 MB) share the same flat (s, c) memory order, so a tile of 2 batches
views as [128 partitions, 8192 free] f32 with W replicated twice along
partitions.  Each iteration: one 4 MB contiguous DMA, one in-place fp32
tensor_tensor multiply on DVE, one grouped tensor_reduce over the
innermost 64 channels -> [128, 128], one 64 KB output DMA.
"""

import sys
from contextlib import ExitStack

import numpy as np

for _p in ("/opt/trn_rl_repo", "/root/.axon_site/_ro/trn_rl_repo"):
    if _p not in sys.path:
        sys.path.insert(0, _p)

import concourse.bacc as bacc
import concourse.mybir as mybir
import concourse.tile as tile
from concourse.bass_utils import run_bass_kernel_spmd

B, S, C = 256, 8192, 64
NCORES = 8
BPC = B // NCORES          # 32 batches per core
BPT = 2                    # batches per tile
NT = BPC // BPT            # 16 tiles per core
P = 128
FREE = BPT * S * C // P    # 8192 f32 per partition line
OUT_FREE = BPT * S // P    # 128 outputs per partition line

_cache = {}

# Free-axis split of the elementwise multiply: columns [0, K_DVE) go to the
# Vector engine, [K_DVE, FREE) to GpSimd.  fp32 TT is 1x on DVE (~1.04
# ns/elem-lane) and ~2.2 ns/elem-lane on GpSimd; DVE also owns the grouped
# reduce (8192 elems/lane/tile at 1x).  K_DVE = 3584 balances both at ~12
# us/tile, under the ~14 us/tile DMA floor.  Must be a multiple of 64.
K_DVE = 3584


def _build():
    nc = bacc.Bacc("TRN2", debug=False, target_bir_lowering=False)
    x = nc.dram_tensor("x", [BPC * S * C], mybir.dt.float32, kind="ExternalInput").ap()
    w = nc.dram_tensor("w", [S * C], mybir.dt.float32, kind="ExternalInput").ap()
    out = nc.dram_tensor("out", [BPC * S], mybir.dt.float32, kind="ExternalOutput").ap()

    x_v = x.rearrange("(i p f) -> i p f", i=NT, p=P)      # [16, 128, 8192]
    w_v = w.rearrange("(p f) -> p f", p=P // 2)           # [64, 8192]
    o_v = out.rearrange("(i p j) -> i p j", i=NT, p=P)    # [16, 128, 128]

    with tile.TileContext(nc) as tc, ExitStack() as ctx:
        xp = ctx.enter_context(tc.tile_pool(name="xp", bufs=4))
        wp = ctx.enter_context(tc.tile_pool(name="wp", bufs=1))
        op = ctx.enter_context(tc.tile_pool(name="op", bufs=4))

        # W tile [128, 8192]: W viewed [64, 8192], replicated on both
        # partition halves so it lines up with the 2-batch x tiles.  Loaded
        # in column chunks (ACT HWDGE ring) so tile 0's chunked multiplies
        # can start after ~1.5 MB of traffic instead of waiting out the
        # full 8 MB W+x0 fill.
        wt = wp.tile([P, FREE], mybir.dt.float32)
        nc.scalar.dma_start(wt[0 : P // 2, :], w_v[:, :])
        nc.scalar.dma_start(wt[P // 2 : P, :], w_v[:, :])

        for i in range(NT):
            xt = xp.tile([P, FREE], mybir.dt.float32)
            nc.sync.dma_start(xt[:], x_v[i])
            nc.vector.tensor_mul(xt[:, :K_DVE], xt[:, :K_DVE], wt[:, :K_DVE])
            nc.gpsimd.tensor_mul(xt[:, K_DVE:], xt[:, K_DVE:], wt[:, K_DVE:])
            ot = op.tile([P, OUT_FREE], mybir.dt.float32)
            # Split reduce: the low half reads only DVE-mul'd columns
            # (K_DVE >= FREE/2), so it never stalls on the GpSimd multiply.
            x3 = xt[:].rearrange("p (j c) -> p j c", c=C)
            o2 = ot[:].rearrange("p (h j) -> p h j", h=2)
            nc.vector.tensor_reduce(
                o2[:, 0], x3[:, : OUT_FREE // 2], axis=mybir.AxisListType.X,
                op=mybir.AluOpType.add,
            )
            nc.vector.tensor_reduce(
                o2[:, 1], x3[:, OUT_FREE // 2 :], axis=mybir.AxisListType.X,
                op=mybir.AluOpType.add,
            )
            nc.sync.dma_start(o_v[i], ot[:])

    nc.compile()
    return nc


def _get_nc():
    if "nc" not in _cache:
        _cache["nc"] = _build()
    return _cache["nc"]


def run_sharded(x, W, **spmd_kwargs):
    """Shard, run on 8 cores, gather. Returns (out[B, S], BassKernelResults)."""
    nc = _get_nc()
    xf = np.ascontiguousarray(x, dtype=np.float32).reshape(NCORES, BPC * S * C)
    wf = np.ascontiguousarray(W, dtype=np.float32).reshape(S * C)
    in_maps = [{"x": xf[i], "w": wf} for i in range(NCORES)]
    r = run_bass_kernel_spmd(nc, in_maps, list(range(NCORES)), **spmd_kwargs)
    out = np.concatenate(
        [np.asarray(r.results[i]["out"]).reshape(BPC, S) for i in range(NCORES)],
        axis=0,
    )
    return out, r


def kernel(x, W):
    out, _ = run_sharded(x, W)
    return out[..., None].astype(np.float32)

